# revision 4
# baseline (speedup 1.0000x reference)
"""Complex coherency loss, distributed over 8 TRN2 NeuronCores.

Data-parallel over batch: core b computes the partial coherency sum for
batch element b; the host sums the 8x128 partials and finishes the mean.

Per-core math (shard [C=64, L=16384] viewed as [P=128, N=8192], partition
p = 2c + h where h = l // 8192, n = l % 8192):
  - DVE: 4 cross products (pr*tr, pi*ti, pi*tr, pr*ti) -> bf16
  - ACT: 4 squares (pr^2, pi^2, tr^2, ti^2)            -> bf16
  - PE : per 512-chunk, 8 matmuls against [128, 8] +/-1 selector weights
         channel-reduce all 8 product tensors into one [8, 512] PSUM
         region (rows 2q+h: q in {ptr, pti, pa, ta})
  - ACT copies PSUM -> SBUF, DMA scatters to a [4, 16512] DRAM staging
    buffer laid out as staging[q, l].
  - Tail: strided DMA reshapes staging rows into a [128, 4*132] halo tile
    (partition p holds l = 128p .. 128p+131), 4 shifted adds do the
    k=5 sliding-window sum, then ratio = sqrt((wr^2+wi^2)/(wa*wt)),
    masked at the 4 invalid trailing positions, free-axis reduced to
    [128, 1] and DMA'd out.
"""

import numpy as np
import ml_dtypes

import concourse.bass as bass
import concourse.bacc as bacc
import concourse.mybir as mybir
import concourse.tile as tile
from concourse.bass_utils import run_bass_kernel_spmd

B, C, L = 8, 64, 16384
K = 5
P = 128
N = (C * L) // P          # 8192 free positions per core view
NVALID = L - K + 1        # 16380
GROUPS = 4
FD = N // GROUPS          # 2048
CH = 512                  # matmul moving-dim chunk
STG_W = L + 128           # staging row width (128 zero pad at tail)

F32 = mybir.dt.float32
BF16 = mybir.dt.bfloat16

PROFILE = False
TRACE_DIR = None
LAST_RESULT = None


def _selector_weights() -> np.ndarray:
    """Five [128, 8] weight matrices, packed as [128, 40] bf16.

    Matrix w maps a product tensor into PSUM rows 2q+h (h = p % 2):
      w=0: m1,m2 -> rows 0,1 (ptr, +)    w=1: m3 -> rows 2,3 (pti, +)
      w=2: m4    -> rows 2,3 (pti, -)    w=3: s1,s2 -> rows 4,5 (pa, +)
      w=4: s3,s4 -> rows 6,7 (ta, +)
    """
    w = np.zeros((P, 5 * 8), dtype=np.float32)
    p = np.arange(P)
    h = p % 2
    w[p, 0 * 8 + 0 + h] = 1.0
    w[p, 1 * 8 + 2 + h] = 1.0
    w[p, 2 * 8 + 2 + h] = -1.0
    w[p, 3 * 8 + 4 + h] = 1.0
    w[p, 4 * 8 + 6 + h] = 1.0
    return w.astype(ml_dtypes.bfloat16)


def build_nc() -> bacc.Bacc:
    nc = bacc.Bacc("TRN2", target_bir_lowering=False, debug=False)

    pr_d = nc.dram_tensor("pr", [P, N], F32, kind="ExternalInput").ap()
    pi_d = nc.dram_tensor("pi", [P, N], F32, kind="ExternalInput").ap()
    tr_d = nc.dram_tensor("tr", [P, N], F32, kind="ExternalInput").ap()
    ti_d = nc.dram_tensor("ti", [P, N], F32, kind="ExternalInput").ap()
    out_d = nc.dram_tensor("out", [P, 1], F32, kind="ExternalOutput").ap()
    w_d = nc.inline_tensor(_selector_weights(), name="selw").ap()

    with tile.TileContext(nc) as tc:
        with (
            tc.tile_pool(name="consts", bufs=1) as consts,
            tc.tile_pool(name="ins", bufs=3) as ins,
            tc.tile_pool(name="prods", bufs=2) as prods,
            tc.tile_pool(name="drains", bufs=2) as drains,
            tc.tile_pool(name="fin", bufs=1) as fin,
            tc.tile_pool(name="psum", bufs=2, space="PSUM") as psum,
            tc.tile_pool(name="dram", bufs=1, space="DRAM") as dram,
        ):
            w_sb = consts.tile([P, 5 * 8], BF16)
            nc.sync.dma_start(w_sb[:, :], w_d)

            stg = dram.tile([4, STG_W], F32)

            # Zero the staging tail so halo reads past L are defined.
            zeros = consts.tile([1, 4 * (STG_W - L)], F32)
            nc.vector.memset(zeros[:, :], 0.0)
            nc.sync.dma_start(stg[:, L:STG_W], zeros[:, :])

            stg_main = stg[:, 0:L].rearrange("q (h n) -> q h n", h=2)

            mm_plan = [  # (weight idx, product slot, start, stop)
                (0, 0, True, False),   # m1 = pr*tr
                (0, 1, False, False),  # m2 = pi*ti
                (1, 2, False, False),  # m3 = pi*tr
                (2, 3, False, False),  # m4 = pr*ti (negative weights)
                (3, 4, False, False),  # s1 = pr^2
                (3, 5, False, False),  # s2 = pi^2
                (4, 6, False, False),  # s3 = tr^2
                (4, 7, False, True),   # s4 = ti^2
            ]

            for g in range(GROUPS):
                sl = slice(g * FD, (g + 1) * FD)
                t_pr = ins.tile([P, FD], F32)
                t_pi = ins.tile([P, FD], F32)
                t_tr = ins.tile([P, FD], F32)
                t_ti = ins.tile([P, FD], F32)
                nc.sync.dma_start(t_pr[:, :], pr_d[:, sl])
                nc.sync.dma_start(t_pi[:, :], pi_d[:, sl])
                nc.sync.dma_start(t_tr[:, :], tr_d[:, sl])
                nc.sync.dma_start(t_ti[:, :], ti_d[:, sl])

                m1 = prods.tile([P, FD], BF16)
                m2 = prods.tile([P, FD], BF16)
                m3 = prods.tile([P, FD], BF16)
                m4 = prods.tile([P, FD], BF16)
                nc.vector.tensor_mul(m1[:, :], t_pr[:, :], t_tr[:, :])
                nc.vector.tensor_mul(m2[:, :], t_pi[:, :], t_ti[:, :])
                nc.vector.tensor_mul(m3[:, :], t_pi[:, :], t_tr[:, :])
                nc.vector.tensor_mul(m4[:, :], t_pr[:, :], t_ti[:, :])

                s1 = prods.tile([P, FD], BF16)
                s2 = prods.tile([P, FD], BF16)
                s3 = prods.tile([P, FD], BF16)
                s4 = prods.tile([P, FD], BF16)
                nc.scalar.square(s1[:, :], t_pr[:, :])
                nc.scalar.square(s2[:, :], t_pi[:, :])
                nc.scalar.square(s3[:, :], t_tr[:, :])
                nc.scalar.square(s4[:, :], t_ti[:, :])

                prod_tiles = [m1, m2, m3, m4, s1, s2, s3, s4]

                ps = psum.tile([8, FD], F32)
                for widx, pslot, start, stop in mm_plan:
                    prod = prod_tiles[pslot]
                    lhsT = w_sb[:, widx * 8:(widx + 1) * 8]
                    for k in range(FD // CH):
                        ks = slice(k * CH, (k + 1) * CH)
                        nc.tensor.matmul(
                            ps[:, ks], lhsT, prod[:, ks],
                            start=start, stop=stop,
                        )

                dr = drains.tile([8, FD], F32)
                nc.scalar.activation(
                    dr[:, :], ps[:, :], mybir.ActivationFunctionType.Copy
                )
                nc.sync.dma_start(stg_main[:, :, sl], dr[:, :])

            # ---- tail: winsum + ratio + reduce ----
            halo = fin.tile([P, 4 * 132], F32)
            halo_r = halo.rearrange("p (q j) -> p q j", q=4)
            src_main = stg[:, 0:L].rearrange("q (p n) -> p q n", p=P)
            src_ext = stg[:, 128:STG_W].rearrange("q (p j) -> p q j", j=P)
            nc.sync.dma_start(halo_r[:, :, 0:128], src_main)
            nc.sync.dma_start(halo_r[:, :, 128:132], src_ext[:, :, 0:4])

            w = fin.tile([P, 4 * 128], F32)
            w_r = w.rearrange("p (q f) -> p q f", q=4)
            nc.vector.tensor_add(
                w_r, halo_r[:, :, 0:128], halo_r[:, :, 1:129]
            )
            for j in (2, 3, 4):
                nc.vector.tensor_add(w_r, w_r, halo_r[:, :, j:j + 128])

            wr = w[:, 0:128]
            wi = w[:, 128:256]
            wa = w[:, 256:384]
            wt = w[:, 384:512]

            n2 = fin.tile([P, 128], F32)
            t2 = fin.tile([P, 128], F32)
            nc.vector.tensor_mul(n2[:, :], wr, wr)
            nc.vector.tensor_mul(t2[:, :], wi, wi)
            nc.vector.tensor_add(n2[:, :], n2[:, :], t2[:, :])

            d2 = fin.tile([P, 128], F32)
            nc.vector.tensor_mul(d2[:, :], wa, wt)
            rd = fin.tile([P, 128], F32)
            nc.vector.reciprocal(rd[:, :], d2[:, :])
            nc.vector.tensor_mul(n2[:, :], n2[:, :], rd[:, :])

            # positions l >= 16380 are invalid (windows cross the end);
            # DVE can't address a lone partition 127, so mask via DMA
            nc.sync.dma_start(n2[127:128, 124:128], zeros[0:1, 0:4])

            sq = fin.tile([P, 128], F32)
            acc = fin.tile([P, 1], F32)
            nc.scalar.activation(
                sq[:, :], n2[:, :], mybir.ActivationFunctionType.Sqrt,
                accum_out=acc[:, :],
            )
            nc.sync.dma_start(out_d, acc[:, :])

    nc.compile()
    return nc


_NC = None


def _get_nc() -> bacc.Bacc:
    global _NC
    if _NC is None:
        _NC = build_nc()
    return _NC


def kernel(pred_real, pred_imag, targ_real, targ_imag, filter_size=5):
    global LAST_RESULT
    assert int(filter_size) == K
    nc = _get_nc()

    in_maps = []
    for b in range(B):
        in_maps.append({
            "pr": np.ascontiguousarray(pred_real[b], dtype=np.float32).reshape(P, N),
            "pi": np.ascontiguousarray(pred_imag[b], dtype=np.float32).reshape(P, N),
            "tr": np.ascontiguousarray(targ_real[b], dtype=np.float32).reshape(P, N),
            "ti": np.ascontiguousarray(targ_imag[b], dtype=np.float32).reshape(P, N),
        })

    kwargs = {}
    if PROFILE:
        kwargs = dict(trace=True)
        if TRACE_DIR is not None:
            import os
            os.makedirs(TRACE_DIR, exist_ok=True)
            kwargs["tmpdir"] = TRACE_DIR
    res = run_bass_kernel_spmd(nc, in_maps, core_ids=list(range(B)), **kwargs)
    LAST_RESULT = res

    total = 0.0
    for r in res.results:
        total += float(np.asarray(r["out"], dtype=np.float64).sum())
    coh = total / (B * NVALID)
    return np.float32(1.0 - coh)


# revision 7
# speedup vs baseline: 1.0911x; 1.0911x over previous
"""Complex coherency loss, distributed over 8 TRN2 NeuronCores.

Data-parallel over batch: core b computes the partial coherency sum for
batch element b; the host sums the 8x128 partials and finishes the mean.

Per-core math (shard [C=64, L=16384] viewed as [P=128, N=8192], partition
p = 2c + h where h = l // 8192, n = l % 8192):
  - DVE: 4 cross products (pr*tr, pi*ti, pi*tr, pr*ti) -> bf16
  - ACT: 4 squares (pr^2, pi^2, tr^2, ti^2)            -> bf16
  - PE : per 512-chunk, 8 matmuls against [128, 8] +/-1 selector weights
         channel-reduce all 8 product tensors into one [8, 512] PSUM
         region (rows 2q+h: q in {ptr, pti, pa, ta})
  - ACT copies PSUM -> SBUF, DMA scatters to a [4, 16512] DRAM staging
    buffer laid out as staging[q, l].
  - Tail: strided DMA reshapes staging rows into a [128, 4*132] halo tile
    (partition p holds l = 128p .. 128p+131), 4 shifted adds do the
    k=5 sliding-window sum, then ratio = sqrt((wr^2+wi^2)/(wa*wt)),
    masked at the 4 invalid trailing positions, free-axis reduced to
    [128, 1] and DMA'd out.
"""

import numpy as np
import ml_dtypes

import concourse.bass as bass
import concourse.bacc as bacc
import concourse.mybir as mybir
import concourse.tile as tile
from concourse.bass_utils import run_bass_kernel_spmd

B, C, L = 8, 64, 16384
K = 5
P = 128
N = (C * L) // P          # 8192 free positions per core view
NVALID = L - K + 1        # 16380
GROUPS = 4
FD = N // GROUPS          # 2048
CH = 512                  # matmul moving-dim chunk
STG_W = L + 128           # staging row width (128 zero pad at tail)

F32 = mybir.dt.float32
BF16 = mybir.dt.bfloat16

PROFILE = False
TRACE_DIR = None
LAST_RESULT = None


def _selector_weights() -> np.ndarray:
    """Five [128, 8] weight matrices, packed as [128, 40] bf16.

    Matrix w maps a product tensor into PSUM rows 2q+h (h = p % 2):
      w=0: m1,m2 -> rows 0,1 (ptr, +)    w=1: m3 -> rows 2,3 (pti, +)
      w=2: m4    -> rows 2,3 (pti, -)    w=3: s1,s2 -> rows 4,5 (pa, +)
      w=4: s3,s4 -> rows 6,7 (ta, +)
    """
    w = np.zeros((P, 5 * 8), dtype=np.float32)
    p = np.arange(P)
    h = p % 2
    w[p, 0 * 8 + 0 + h] = 1.0
    w[p, 1 * 8 + 2 + h] = 1.0
    w[p, 2 * 8 + 2 + h] = -1.0
    w[p, 3 * 8 + 4 + h] = 1.0
    w[p, 4 * 8 + 6 + h] = 1.0
    return w.astype(ml_dtypes.bfloat16)


def build_nc() -> bacc.Bacc:
    nc = bacc.Bacc("TRN2", target_bir_lowering=False, debug=False)

    pr_d = nc.dram_tensor("pr", [P, N], F32, kind="ExternalInput").ap()
    pi_d = nc.dram_tensor("pi", [P, N], F32, kind="ExternalInput").ap()
    tr_d = nc.dram_tensor("tr", [P, N], F32, kind="ExternalInput").ap()
    ti_d = nc.dram_tensor("ti", [P, N], F32, kind="ExternalInput").ap()
    out_d = nc.dram_tensor("out", [P, 1], F32, kind="ExternalOutput").ap()
    w_d = nc.inline_tensor(_selector_weights(), name="selw").ap()

    with tile.TileContext(nc) as tc:
        with (
            tc.tile_pool(name="consts", bufs=1) as consts,
            tc.tile_pool(name="ins", bufs=3) as ins,
            tc.tile_pool(name="prods", bufs=2) as prods,
            tc.tile_pool(name="drains", bufs=2) as drains,
            tc.tile_pool(name="fin", bufs=1) as fin,
            tc.tile_pool(name="psum", bufs=2, space="PSUM") as psum,
            tc.tile_pool(name="dram", bufs=1, space="DRAM") as dram,
        ):
            w_sb = consts.tile([P, 5 * 8], BF16)
            nc.sync.dma_start(w_sb[:, :], w_d)

            stg = dram.tile([4, STG_W], F32)

            # Zero the staging tail so halo reads past L are defined.
            zeros = consts.tile([1, 4 * (STG_W - L)], F32)
            nc.vector.memset(zeros[:, :], 0.0)
            nc.sync.dma_start(stg[:, L:STG_W], zeros[:, :])

            # Pre-warm the Sqrt activation table so the ~3us lazy table
            # load + drain doesn't land on the serial tail.
            warm = consts.tile([P, 1], F32)
            nc.vector.memset(warm[:, :], 1.0)
            nc.scalar.sqrt(warm[:, :], warm[:, :])

            # Validity mask: positions l >= 16380 (p=127, f>=124) are
            # windows crossing the end of the signal.
            mask = consts.tile([P, 128], F32)
            nc.vector.memset(mask[:, :], 1.0)
            nc.sync.dma_start(mask[127:128, 124:128], zeros[0:1, 0:4])

            stg_main = stg[:, 0:L].rearrange("q (h n) -> q h n", h=2)

            mm_plan = [  # (weight idx, product slot, start, stop)
                (0, 0, True, False),   # m1 = pr*tr
                (0, 1, False, False),  # m2 = pi*ti
                (1, 2, False, False),  # m3 = pi*tr
                (2, 3, False, False),  # m4 = pr*ti (negative weights)
                (3, 4, False, False),  # s1 = pr^2
                (3, 5, False, False),  # s2 = pi^2
                (4, 6, False, False),  # s3 = tr^2
                (4, 7, False, True),   # s4 = ti^2
            ]

            for g in range(GROUPS):
                sl = slice(g * FD, (g + 1) * FD)
                # gpsimd (SWDGE) DMAs cast f32 -> bf16 in the datapath,
                # unlocking the DVE/ACT 2x modes on 16-bit operands
                t_pr = ins.tile([P, FD], BF16)
                t_pi = ins.tile([P, FD], BF16)
                t_tr = ins.tile([P, FD], BF16)
                t_ti = ins.tile([P, FD], BF16)
                nc.gpsimd.dma_start(t_pr[:, :], pr_d[:, sl])
                nc.gpsimd.dma_start(t_tr[:, :], tr_d[:, sl])
                nc.gpsimd.dma_start(t_pi[:, :], pi_d[:, sl])
                nc.gpsimd.dma_start(t_ti[:, :], ti_d[:, sl])

                m1 = prods.tile([P, FD], BF16)
                m2 = prods.tile([P, FD], BF16)
                m3 = prods.tile([P, FD], BF16)
                m4 = prods.tile([P, FD], BF16)
                nc.vector.tensor_mul(m1[:, :], t_pr[:, :], t_tr[:, :])
                nc.vector.tensor_mul(m2[:, :], t_pi[:, :], t_ti[:, :])
                nc.vector.tensor_mul(m3[:, :], t_pi[:, :], t_tr[:, :])
                nc.vector.tensor_mul(m4[:, :], t_pr[:, :], t_ti[:, :])

                s1 = prods.tile([P, FD], BF16)
                s2 = prods.tile([P, FD], BF16)
                s3 = prods.tile([P, FD], BF16)
                s4 = prods.tile([P, FD], BF16)
                nc.scalar.square(s1[:, :], t_pr[:, :])
                nc.scalar.square(s2[:, :], t_pi[:, :])
                nc.scalar.square(s3[:, :], t_tr[:, :])
                nc.scalar.square(s4[:, :], t_ti[:, :])

                prod_tiles = [m1, m2, m3, m4, s1, s2, s3, s4]

                ps = psum.tile([8, FD], F32)
                for widx, pslot, start, stop in mm_plan:
                    prod = prod_tiles[pslot]
                    lhsT = w_sb[:, widx * 8:(widx + 1) * 8]
                    for k in range(FD // CH):
                        ks = slice(k * CH, (k + 1) * CH)
                        nc.tensor.matmul(
                            ps[:, ks], lhsT, prod[:, ks],
                            start=start, stop=stop,
                        )

                dr = drains.tile([8, FD], F32)
                nc.scalar.activation(
                    dr[:, :], ps[:, :], mybir.ActivationFunctionType.Copy
                )
                nc.sync.dma_start(stg_main[:, :, sl], dr[:, :])

            # ---- tail: winsum + ratio + reduce ----
            halo = fin.tile([P, 4 * 132], F32)
            halo_r = halo.rearrange("p (q j) -> p q j", q=4)
            src_main = stg[:, 0:L].rearrange("q (p n) -> p q n", p=P)
            src_ext = stg[:, 128:STG_W].rearrange("q (p j) -> p q j", j=P)
            nc.sync.dma_start(halo_r[:, :, 0:128], src_main)
            nc.sync.dma_start(halo_r[:, :, 128:132], src_ext[:, :, 0:4])

            w = fin.tile([P, 4 * 128], F32)
            w_r = w.rearrange("p (q f) -> p q f", q=4)
            nc.vector.tensor_add(
                w_r, halo_r[:, :, 0:128], halo_r[:, :, 1:129]
            )
            for j in (2, 3, 4):
                nc.vector.tensor_add(w_r, w_r, halo_r[:, :, j:j + 128])

            wr = w[:, 0:128]
            wi = w[:, 128:256]
            wa = w[:, 256:384]
            wt = w[:, 384:512]

            n2 = fin.tile([P, 128], F32)
            t2 = fin.tile([P, 128], F32)
            nc.vector.tensor_mul(n2[:, :], wr, wr)
            nc.vector.tensor_mul(t2[:, :], wi, wi)
            nc.vector.tensor_add(n2[:, :], n2[:, :], t2[:, :])

            d2 = fin.tile([P, 128], F32)
            nc.vector.tensor_mul(d2[:, :], wa, wt)
            rd = fin.tile([P, 128], F32)
            nc.vector.reciprocal(rd[:, :], d2[:, :])
            nc.vector.tensor_mul(n2[:, :], n2[:, :], rd[:, :])
            nc.vector.tensor_mul(n2[:, :], n2[:, :], mask[:, :])

            sq = fin.tile([P, 128], F32)
            acc = fin.tile([P, 1], F32)
            nc.scalar.activation(
                sq[:, :], n2[:, :], mybir.ActivationFunctionType.Sqrt,
                accum_out=acc[:, :],
            )
            nc.sync.dma_start(out_d, acc[:, :])

    nc.compile()
    return nc


_NC = None


def _get_nc() -> bacc.Bacc:
    global _NC
    if _NC is None:
        _NC = build_nc()
    return _NC


def kernel(pred_real, pred_imag, targ_real, targ_imag, filter_size=5):
    global LAST_RESULT
    assert int(filter_size) == K
    nc = _get_nc()

    in_maps = []
    for b in range(B):
        in_maps.append({
            "pr": np.ascontiguousarray(pred_real[b], dtype=np.float32).reshape(P, N),
            "pi": np.ascontiguousarray(pred_imag[b], dtype=np.float32).reshape(P, N),
            "tr": np.ascontiguousarray(targ_real[b], dtype=np.float32).reshape(P, N),
            "ti": np.ascontiguousarray(targ_imag[b], dtype=np.float32).reshape(P, N),
        })

    kwargs = {}
    if PROFILE:
        kwargs = dict(trace=True)
        if TRACE_DIR is not None:
            import os
            os.makedirs(TRACE_DIR, exist_ok=True)
            kwargs["tmpdir"] = TRACE_DIR
    res = run_bass_kernel_spmd(nc, in_maps, core_ids=list(range(B)), **kwargs)
    LAST_RESULT = res

    total = 0.0
    for r in res.results:
        total += float(np.asarray(r["out"], dtype=np.float64).sum())
    coh = total / (B * NVALID)
    return np.float32(1.0 - coh)
